# revision 1
# baseline (speedup 1.0000x reference)
"""Nearest-color-distance loss on 8 TRN2 NeuronCores.

loss = mean_i min_j ||x_i - p_j||_2,  x: (131072, 3), p: (128, 3).

Per core (16384 colors): d2(i,j) = ||p_j||^2 - 2 x_i.p_j + ||x_i||^2
computed entirely inside the PE via 5-row packings (x_ch, 1, ||x||^2
against -2p_ch, ||p||^2, 1). Two layouts run interleaved so no single
reduction engine gates the loop:
 - 27 "bd" groups: 4 color-chunks block-diagonal (K=20) per matmul,
   colors on PSUM partitions; DVE min-reduces pairs of groups over the
   palette (free) axis (13 pairs + 1 single).
 - 5 "sw" groups: palette stationary (K=5), colors moving; palette on
   PSUM partitions; ACT negate-copies PSUM->SBUF and GpSimd max-reduces
   over the partition (C) axis (no min op -> negate trick).
The gpsimd PartitionAllReduce library load takes ~7.6us in the
background, so no DMA is placed on the gpsimd queue (LIBRARY_RELOAD
issues right after pool init). p20/xt1 are staged first and small so
the bd pipeline starts ASAP; outputs are split so result DMAs overlap
the tails of the reduce chains. Raw min-d2 go back to the host, which
does sqrt/clamp/mean in f64, plus layout + centering prep.
"""

import sys

sys.path.insert(0, "/opt/trn_rl_repo")

import numpy as np

import concourse.bass as bass
import concourse.bass_isa as bass_isa
import concourse.tile as tile
from concourse import bacc, mybir
from concourse.alu_op_type import AluOpType
from concourse.bass_utils import run_bass_kernel_spmd

N_CORES = 8
N = 131072
NPC = N // N_CORES  # 16384 colors per core
M = 128  # palette size
BD = 27  # block-diagonal groups of 512 colors (13 pairs + 1 single)
SW = 5  # swapped-layout groups (ACT+GpSimd-consumed)
NBD = BD * 512  # 13824 colors via bd path
NSW = NPC - NBD  # 2560 colors via sw path
WB = 128 * BD  # 3456 xt columns
F32 = mybir.dt.float32
F32R = mybir.dt.float32r
AF = mybir.ActivationFunctionType

MM_DT = F32R  # full-rate PE dtype; flip to F32 if precision fails


def build_nc():
    nc = bacc.Bacc(
        "TRN2",
        target_bir_lowering=False,
        debug=False,
        enable_asserts=False,
        num_devices=N_CORES,
    )
    aux1_d = nc.dram_tensor("aux1", [5, 1152], F32, kind="ExternalInput").ap()
    aux2_d = nc.dram_tensor("aux2", [5, NSW - 1024], F32, kind="ExternalInput").ap()
    p20_d = nc.dram_tensor("p20", [20, 512], F32, kind="ExternalInput").ap()
    xt1_d = nc.dram_tensor("xt1", [20, 512], F32, kind="ExternalInput").ap()
    xt2a_d = nc.dram_tensor("xt2a", [20, 1280], F32, kind="ExternalInput").ap()
    xt2b_d = nc.dram_tensor("xt2b", [20, WB - 1792], F32, kind="ExternalInput").ap()
    minva_d = nc.dram_tensor("minva", [128, 88], F32, kind="ExternalOutput").ap()
    minvb_d = nc.dram_tensor("minvb", [128, 20], F32, kind="ExternalOutput").ap()
    minr1_d = nc.dram_tensor("minr1", [1, 1536], F32, kind="ExternalOutput").ap()
    minr2_d = nc.dram_tensor("minr2", [1, 1024], F32, kind="ExternalOutput").ap()

    with tile.TileContext(nc) as tc:
        with (
            tc.tile_pool(name="sb", bufs=1) as sb,
            tc.tile_pool(name="cp", bufs=4) as cpp,
            tc.tile_pool(name="pp", bufs=3, space=bass.MemorySpace.PSUM) as pp,
            tc.tile_pool(name="pw", bufs=2, space=bass.MemorySpace.PSUM) as pw,
        ):
            aux1 = sb.tile([5, 1152], MM_DT)
            aux2 = sb.tile([5, NSW - 1024], MM_DT)
            p20t = sb.tile([20, 512], MM_DT)
            xt1 = sb.tile([20, 512], MM_DT)
            xt2a = sb.tile([20, 1280], MM_DT)
            xt2b = sb.tile([20, WB - 1792], MM_DT)
            minva = sb.tile([128, 88], F32)
            minvb = sb.tile([128, 20], F32)
            allra = sb.tile([128, 1536], F32)
            allrb = sb.tile([128, 1024], F32)

            nc.gpsimd.dma_start(aux1[:], aux1_d.bitcast(MM_DT))
            nc.scalar.dma_start(p20t[:], p20_d.bitcast(MM_DT))
            nc.scalar.dma_start(aux2[:], aux2_d.bitcast(MM_DT))
            nc.sync.dma_start(xt1[:], xt1_d.bitcast(MM_DT))
            nc.sync.dma_start(xt2a[:], xt2a_d.bitcast(MM_DT))
            nc.sync.dma_start(xt2b[:], xt2b_d.bitcast(MM_DT))
            pal5 = aux1[:, 0:128]
            p20 = p20t[:]

            def bd_src(g):
                if g < 4:
                    return xt1[:, 128 * g : 128 * (g + 1)]
                if g < 14:
                    return xt2a[:, 128 * (g - 4) : 128 * (g - 3)]
                return xt2b[:, 128 * (g - 14) : 128 * (g - 13)]

            def sw_one(s):
                mov = (
                    aux1[:, 128 + 512 * s : 640 + 512 * s]
                    if s < 2
                    else aux2[:, 512 * (s - 2) : 512 * (s - 1)]
                )
                d_ps = pw.tile([128, 512], F32)
                nc.tensor.matmul(d_ps[:], pal5[:], mov, start=True, stop=True)
                cp = cpp.tile([128, 512], F32)
                nc.scalar.mul(cp[:], d_ps[:], -1.0)
                dst = (
                    allra[:, bass.ts(s, 512)]
                    if s < 3
                    else allrb[:, bass.ts(s - 3, 512)]
                )
                nc.gpsimd.partition_all_reduce(
                    dst,
                    cp[:],
                    channels=128,
                    reduce_op=bass_isa.ReduceOp.max,
                )

            def bd_pair(p):
                d_ps = pp.tile([128, 1024], F32)
                for h in range(2):
                    nc.tensor.matmul(
                        d_ps[:, 512 * h : 512 * (h + 1)],
                        bd_src(2 * p + h),
                        p20,
                        start=True,
                        stop=True,
                    )
                out = (
                    minva[:, 8 * p : 8 * p + 8]
                    if p < 11
                    else minvb[:, 8 * (p - 11) : 8 * (p - 11) + 8]
                )
                nc.vector.tensor_reduce(
                    out,
                    d_ps[:].rearrange("p (c j) -> p c j", j=128),
                    axis=mybir.AxisListType.X,
                    op=AluOpType.min,
                )

            def bd_single():
                d_ps = pp.tile([128, 1024], F32)
                nc.tensor.matmul(
                    d_ps[:, 0:512], bd_src(26), p20, start=True, stop=True
                )
                nc.vector.tensor_reduce(
                    minvb[:, 16:20],
                    d_ps[:, 0:512].rearrange("p (c j) -> p c j", j=128),
                    axis=mybir.AxisListType.X,
                    op=AluOpType.min,
                )

            sw_one(0)
            sw_one(1)
            bd_pair(0)
            bd_pair(1)
            sw_one(2)
            bd_pair(2)
            sw_one(3)
            bd_pair(3)
            sw_one(4)
            for p in range(4, 13):
                bd_pair(p)
            bd_single()

            nc.scalar.dma_start(minr1_d[:], allra[0:1, :])
            nc.scalar.dma_start(minr2_d[:], allrb[0:1, :])
            nc.sync.dma_start(minva_d[:], minva[:])
            nc.sync.dma_start(minvb_d[:], minvb[:])

    nc.compile()
    return nc


def prep_inputs(output_colors, target_palette):
    pal = np.asarray(target_palette, dtype=np.float32)
    mu = pal.mean(axis=0)
    pp = pal - mu  # (128, 3) centered palette
    pn = (pp * pp).sum(axis=1)  # (128,)

    p20 = np.zeros((20, 512), dtype=np.float32)
    for c in range(4):
        p20[5 * c : 5 * c + 3, 128 * c : 128 * (c + 1)] = -2.0 * pp.T
        p20[5 * c + 3, 128 * c : 128 * (c + 1)] = pn
        p20[5 * c + 4, 128 * c : 128 * (c + 1)] = 1.0

    x = np.asarray(output_colors, dtype=np.float32) - mu
    in_maps = []
    for k in range(N_CORES):
        xs = x[k * NPC : (k + 1) * NPC]  # (16384, 3)
        xn2 = (xs * xs).sum(axis=1)  # (16384,)

        xb = xs[:NBD].reshape(BD, 4, 128, 3)  # [g, c, i, ch]
        nb = xn2[:NBD].reshape(BD, 4, 128)
        xt = np.empty((4, 5, BD, 128), dtype=np.float32)  # [c, row, g, i]
        xt[:, 0:3] = xb.transpose(1, 3, 0, 2)
        xt[:, 3] = 1.0
        xt[:, 4] = nb.transpose(1, 0, 2)
        xt = xt.reshape(20, WB)

        xsw = np.empty((5, NSW), dtype=np.float32)
        xsw[0:3] = xs[NBD:].T
        xsw[3] = 1.0
        xsw[4] = xn2[NBD:]
        aux1 = np.empty((5, 1152), dtype=np.float32)
        aux1[0:3, 0:128] = -2.0 * pp.T
        aux1[3, 0:128] = pn
        aux1[4, 0:128] = 1.0
        aux1[:, 128:] = xsw[:, 0:1024]

        in_maps.append(
            {
                "aux1": aux1,
                "aux2": np.ascontiguousarray(xsw[:, 1024:]),
                "p20": p20,
                "xt1": np.ascontiguousarray(xt[:, :512]),
                "xt2a": np.ascontiguousarray(xt[:, 512:1792]),
                "xt2b": np.ascontiguousarray(xt[:, 1792:]),
            }
        )
    return in_maps


_NC_CACHE = {}


def get_nc():
    if "nc" not in _NC_CACHE:
        _NC_CACHE["nc"] = build_nc()
    return _NC_CACHE["nc"]


def kernel(output_colors=None, target_palette=None, _trace=False, **_):
    nc = get_nc()
    in_maps = prep_inputs(output_colors, target_palette)
    res = run_bass_kernel_spmd(
        nc, in_maps, core_ids=list(range(N_CORES)), trace=_trace
    )
    total = np.float64(0.0)
    for r in res.results:
        mv = np.concatenate([r["minva"], r["minvb"]], axis=1)
        mr = np.concatenate([r["minr1"], r["minr2"]], axis=1)
        d2b = np.maximum(mv.astype(np.float64), 0.0)
        d2s = np.maximum(-mr.astype(np.float64), 0.0)
        total += np.sqrt(d2b).sum() + np.sqrt(d2s).sum()
    out = np.array(total / N, dtype=np.float32)
    if _trace:
        kernel._last_results = res
    return out


if __name__ == "__main__":
    rng = np.random.default_rng(0)
    oc = rng.random((N, 3), dtype=np.float32)
    tp = rng.random((M, 3), dtype=np.float32)
    got = kernel(output_colors=oc, target_palette=tp)
    d = oc[:, None, :] - tp[None, :, :]
    want = np.sqrt((d * d).sum(-1)).min(1).mean(dtype=np.float64)
    print("got", got, "want", want, "rel", abs(got - want) / abs(want))

